# revision 15
# baseline (speedup 1.0000x reference)
"""Trainium2 Bass kernel for nn_AutoEncoder (GRU seq2seq + vocab projection + CE loss).

Sharding: vocab-sharded tensor parallelism for the dominant [4032,768]@[768,32000]
GEMM (core c owns Wout rows [c*4000,(c+1)*4000)); the latency-bound GRUs are
replicated on all 8 cores so no per-step collectives are needed.

Device program (per core, identical NEFF):
  1. indirect-DMA embedding gathers + PE transposes -> exT/eyT (E on partitions)
  2. input-gate GEMMs xgT/ygT = WihT.T @ e{x,y}T staged to DRAM (bf16, T layout);
     z-gate gets bias + the packed-sequence mask (+30 where t >= len_b)
  3. encoder GRU, 64 steps, T layout: weights-stationary matmuls accumulate
     h-gates on top of identity-matmul-preloaded xg (r,z); gate math uses only
     tanh (sigmoid(x)=0.5(1+tanh(x/2))) so one ACT table set serves tanh+exp;
     fp32 hidden ring kills bf16 accumulation drift; exact freeze at t>=len via
     z==1.0 saturation
  4. decoder GRU 63 steps, interleaved with the vocab-shard GEMM every 2 steps;
     per 128-row tile: 4 psum quarters of 1000 vocab, evacuated by DVE+ACT,
     exp+rowsum fused into one ACT pass (accum_out), logits DMA'd out fp32
Host: concatenates 8 logit shards, prepends the zero row, combines rowsum-exp
shards and does the O(T*B) cross-entropy scalar math.
"""

import sys

sys.path.insert(0, "/opt/trn_rl_repo")

import numpy as np
import ml_dtypes

import concourse.bass as bass
import concourse.bacc as bacc
import concourse.mybir as mybir
import concourse.tile as tile
from concourse.masks import make_identity
from concourse.bass_utils import run_bass_kernel_spmd

BF16 = mybir.dt.bfloat16
F32 = mybir.dt.float32
I32 = mybir.dt.int32
AF = mybir.ActivationFunctionType
OP = mybir.AluOpType

V, E, H, T, B = 32000, 256, 512, 64, 64
H3 = 3 * H                      # 1536
NCORES = 8
VS = V // NCORES                # 4000 vocab rows per core
NTOK_X = T * B                  # 4096
NTOK_Y = (T - 1) * B            # 4032
KH = H // 128                   # 4
KE = E // 128                   # 2
MG = H3 // 128                  # 12
KF = (H + E) // 128             # 6
NMT = (NTOK_Y + 127) // 128     # 32 output m-tiles (last has 64 rows)
QV = 1000                       # vocab quarter per psum tile (2 banks)
MASK_BIG = 30.0

_CACHE = {}


def _build(has_bout):
    nc = bacc.Bacc()

    xy_ids = nc.declare_dram_parameter("xy_ids", [128, 2 * (NTOK_X // 128)], I32, isOutput=False)
    xlens = nc.declare_dram_parameter("xlens", [128, B], F32, isOutput=False)
    emb_d = nc.declare_dram_parameter("emb_bf", [V, E], BF16, isOutput=False)
    wihT_e = nc.declare_dram_parameter("wihT_enc", [E, H3], BF16, isOutput=False)
    whhT_e = nc.declare_dram_parameter("whhT_enc", [H, H3], BF16, isOutput=False)
    wihT_d = nc.declare_dram_parameter("wihT_dec", [E, H3], BF16, isOutput=False)
    whhT_d = nc.declare_dram_parameter("whhT_dec", [H, H3], BF16, isOutput=False)
    woutT_d = nc.declare_dram_parameter("woutT", [KF * 128, VS], BF16, isOutput=False)
    bias_e = nc.declare_dram_parameter("bias_enc", [H3], F32, isOutput=False)
    bias_dd = nc.declare_dram_parameter("bias_dec", [H3], F32, isOutput=False)
    bout_d = nc.declare_dram_parameter("bout", [VS], F32, isOutput=False)
    logits_out = nc.declare_dram_parameter("logits_out", [NTOK_Y, VS], F32, isOutput=True)
    sumexp_out = nc.declare_dram_parameter("sumexp_out", [NTOK_Y], F32, isOutput=True)

    xgT_dram = nc.dram_tensor("xgT_stage", [128, MG, NTOK_X], BF16)
    ygT_dram = nc.dram_tensor("ygT_stage", [128, MG, NTOK_X], BF16)

    with tile.TileContext(nc) as tc:
        with (
            tc.tile_pool(name="persist", bufs=1) as pp,
            tc.tile_pool(name="state", bufs=1) as sp,
        ):
            ident = pp.tile([128, 128], BF16)
            make_identity(nc, ident[:])

            xy_sb = pp.tile([128, 2 * (NTOK_X // 128)], I32, tag="xyids")
            nc.gpsimd.dma_start(out=xy_sb[:], in_=xy_ids.ap())
            lens128 = pp.tile([128, B], F32, tag="lens128")
            nc.sync.dma_start(out=lens128[:], in_=xlens.ap())

            wihT_e_sb = pp.tile([128, KE, H3], BF16, tag="wihe")
            nc.sync.dma_start(out=wihT_e_sb[:], in_=wihT_e.ap().rearrange("(k p) m -> p k m", p=128))
            wihT_d_sb = pp.tile([128, KE, H3], BF16, tag="wihd")
            nc.sync.dma_start(out=wihT_d_sb[:], in_=wihT_d.ap().rearrange("(k p) m -> p k m", p=128))
            whhT_e_sb = pp.tile([128, KH, H3], BF16, tag="whhe")
            nc.sync.dma_start(out=whhT_e_sb[:], in_=whhT_e.ap().rearrange("(k p) m -> p k m", p=128))
            whhT_d_sb = pp.tile([128, KH, H3], BF16, tag="whhd")
            nc.sync.dma_start(out=whhT_d_sb[:], in_=whhT_d.ap().rearrange("(k p) m -> p k m", p=128))
            woutT_sb = pp.tile([128, KF, VS], BF16, tag="wout")
            nc.sync.dma_start(out=woutT_sb[:], in_=woutT_d.ap().rearrange("(k p) n -> p k n", p=128))
            bias_e_sb = pp.tile([128, MG], F32, tag="biase")
            nc.sync.dma_start(out=bias_e_sb[:], in_=bias_e.ap().rearrange("(m p) -> p m", p=128))
            bias_d_sb = pp.tile([128, MG], F32, tag="biasd")
            nc.sync.dma_start(out=bias_d_sb[:], in_=bias_dd.ap().rearrange("(m p) -> p m", p=128))
            if has_bout:
                bout_f_sb = pp.tile([1, VS], F32, tag="boutf")
                nc.sync.dma_start(out=bout_f_sb[:], in_=bout_d.ap().rearrange("(a b) -> a b", a=1))
                bout_sb = pp.tile([1, VS], BF16, tag="bout")
                nc.vector.tensor_copy(out=bout_sb[:], in_=bout_f_sb[:])
                ones_sb = pp.tile([1, 128], BF16, tag="ones")
                nc.vector.memset(ones_sb[:], 1.0)

            # mask30[p, t*64+b] = 30.0 where t >= len_b (same for all partitions p)
            mask_tb = pp.tile([128, NTOK_X], BF16, tag="mask")
            for t in range(T):
                # mask[:, t*B+b] = (len_b <= t) * 30
                nc.vector.tensor_scalar(
                    out=mask_tb[:, t * B : (t + 1) * B], in0=lens128[:],
                    scalar1=float(t), scalar2=MASK_BIG,
                    op0=OP.is_le, op1=OP.mult,
                )

            # GRU state
            h_f32 = sp.tile([128, KH * B], F32, tag="hf32")
            h_bf = sp.tile([128, KH * B], BF16, tag="hbf")
            nc.vector.memset(h_f32[:], 0.0)
            nc.vector.memset(h_bf[:], 0.0)
            dec_hT = sp.tile([128, KH, NTOK_Y], BF16, tag="dechT")
            eyT_sb = sp.tile([128, KE, NTOK_X], BF16, tag="eyT")
            sumexp_sb = sp.tile([128, NMT], F32, tag="sumexp")

            # ---------- Phase 1+2: gathers, transposes, input-gate GEMMs ----------
            with (
                tc.tile_pool(name="gath", bufs=64) as gp,
                tc.tile_pool(name="tpsum", bufs=2, space="PSUM") as tpp,
                tc.tile_pool(name="exT_pool", bufs=1) as xp,
                tc.tile_pool(name="xgpsum", bufs=2, space="PSUM") as xgp,
                tc.tile_pool(name="xgsb", bufs=3) as xsb,
            ):
                exT_sb = xp.tile([128, KE, NTOK_X], BF16, tag="exT")

                def gather_transpose(ids_sb, dst):
                    for i in range(NTOK_X // 128):
                        g = gp.tile([128, E], BF16, tag="gtile")
                        nc.gpsimd.indirect_dma_start(
                            out=g[:],
                            out_offset=None,
                            in_=emb_d.ap(),
                            in_offset=bass.IndirectOffsetOnAxis(ap=ids_sb[:, i : i + 1], axis=0),
                        )
                        for k in range(KE):
                            ps = tpp.tile([128, 128], BF16, tag="tp")
                            nc.tensor.transpose(out=ps[:], in_=g[:, k * 128 : (k + 1) * 128], identity=ident[:])
                            nc.vector.tensor_copy(out=dst[:, k, i * 128 : (i + 1) * 128], in_=ps[:])

                gather_transpose(xy_sb, exT_sb)
                gather_transpose(xy_sb[:, NTOK_X // 128 :], eyT_sb)

                CH = 512

                def input_gates(wih_sb, src_T, dst_dram, bias_sb, masked):
                    for j in range(MG):
                        for c in range(NTOK_X // CH):
                            ps = xgp.tile([128, CH], F32, tag="xgps")
                            for k in range(KE):
                                nc.tensor.matmul(
                                    out=ps[:],
                                    lhsT=wih_sb[:, k, j * 128 : (j + 1) * 128],
                                    rhs=src_T[:, k, c * CH : (c + 1) * CH],
                                    start=(k == 0),
                                    stop=(k == KE - 1),
                                )
                            ev = xsb.tile([128, CH], BF16, tag="xgev")
                            if masked and 4 <= j < 8:
                                nc.vector.scalar_tensor_tensor(
                                    out=ev[:], in0=ps[:],
                                    scalar=bias_sb[:, j : j + 1],
                                    in1=mask_tb[:, c * CH : (c + 1) * CH],
                                    op0=OP.add, op1=OP.add,
                                )
                            else:
                                nc.vector.tensor_scalar(
                                    out=ev[:], in0=ps[:],
                                    scalar1=bias_sb[:, j : j + 1], scalar2=None,
                                    op0=OP.add,
                                )
                            nc.sync.dma_start(out=dst_dram[:, j, c * CH : (c + 1) * CH], in_=ev[:])

                input_gates(wihT_e_sb, exT_sb, xgT_dram.ap(), bias_e_sb, True)
                input_gates(wihT_d_sb, eyT_sb, ygT_dram.ap(), bias_d_sb, False)

            # ---------- Phase 3+4: GRU loops + fused output GEMM ----------
            with (
                tc.tile_pool(name="grupsum", bufs=2, space="PSUM") as gpp,
                tc.tile_pool(name="gempsum", bufs=4, space="PSUM") as lpp,
                tc.tile_pool(name="xgslice", bufs=4) as xsl,
                tc.tile_pool(name="gates", bufs=2) as gb,
                tc.tile_pool(name="lsb", bufs=3) as lsb,
                tc.tile_pool(name="qsums", bufs=2) as qs,
            ):
                RZ, NN = 8 * B, 4 * B  # 512, 256

                def gru_step(xg_dram, whh_sb, h_in, h_out):
                    """h_in(k)->AP [128,64] bf16; h_out: AP [128,4,64] view (bf16)."""
                    xg = xsl.tile([128, MG, B], BF16, tag="xgsl")
                    nc.sync.dma_start(out=xg[:], in_=xg_dram[:, :, bass.ts(gru_step.t, B)])
                    ps = gpp.tile([128, MG * B], F32, tag="gps")
                    # bank0 = cols 0-511 (r,z), bank1 = cols 512-767 (n).
                    # start=True clears has_written for the WHOLE bank, so
                    # exactly one start per bank (the chronologically first MM).
                    for j in range(8):
                        nc.tensor.matmul(
                            out=ps[:, j * B : (j + 1) * B], lhsT=ident[:], rhs=xg[:, j, :],
                            start=(j == 0), stop=False, skip_group_check=True,
                        )
                    for j in range(MG):
                        for k in range(KH):
                            nc.tensor.matmul(
                                out=ps[:, j * B : (j + 1) * B],
                                lhsT=whh_sb[:, k, j * 128 : (j + 1) * 128],
                                rhs=h_in(k),
                                start=(j == 8 and k == 0),
                                stop=(k == KH - 1),
                                skip_group_check=True,
                            )
                    trz = gb.tile([128, RZ], BF16, tag="trz")
                    nc.scalar.activation(trz[:], ps[:, 0:RZ], AF.Tanh, scale=0.5)
                    pp_t = gb.tile([128, NN], BF16, tag="ppt")
                    nc.vector.scalar_tensor_tensor(
                        out=pp_t[:], in0=trz[:, 0:NN], scalar=1.0, in1=ps[:, RZ : RZ + NN],
                        op0=OP.add, op1=OP.mult,
                    )
                    an_t = gb.tile([128, NN], BF16, tag="ant")
                    nc.vector.scalar_tensor_tensor(
                        out=an_t[:], in0=pp_t[:], scalar=0.5,
                        in1=xg[:, 8:12, :].rearrange("p a b -> p (a b)"),
                        op0=OP.mult, op1=OP.add,
                    )
                    n_t = gb.tile([128, NN], BF16, tag="nt")
                    nc.scalar.activation(n_t[:], an_t[:], AF.Tanh)
                    v_t = gb.tile([128, NN], F32, tag="vt")
                    nc.vector.scalar_tensor_tensor(
                        out=v_t[:], in0=trz[:, NN:RZ], scalar=1.0, in1=h_f32[:],
                        op0=OP.add, op1=OP.mult,
                    )
                    u_t = gb.tile([128, NN], BF16, tag="ut")
                    nc.vector.scalar_tensor_tensor(
                        out=u_t[:], in0=trz[:, NN:RZ], scalar=1.0, in1=n_t[:],
                        op0=OP.subtract, op1=OP.mult,
                    )
                    w_t = gb.tile([128, NN], F32, tag="wt")
                    nc.vector.tensor_tensor(out=w_t[:], in0=v_t[:], in1=u_t[:], op=OP.subtract)
                    nc.vector.tensor_scalar_mul(h_f32[:], w_t[:], 0.5)
                    nc.vector.tensor_copy(
                        out=h_out, in_=h_f32[:].rearrange("p (a b) -> p a b", a=KH)
                    )

                def gemm_mtile(j):
                    rows = min(128, NTOK_Y - j * 128)
                    NQ = 8  # 500-vocab chunks, one psum bank each
                    qsum = qs.tile([128, NQ], F32, tag="qsum")
                    for q in range(NQ):
                        nsl = slice(q * 500, (q + 1) * 500)
                        ps = lpp.tile([128, 500], F32, tag="lps")
                        for k in range(KF):
                            lhsT = (
                                dec_hT[:, k, j * 128 : j * 128 + rows]
                                if k < KH
                                else eyT_sb[:, k - KH, j * 128 : j * 128 + rows]
                            )
                            nc.tensor.matmul(
                                out=ps[:rows, :], lhsT=lhsT, rhs=woutT_sb[:, k, nsl],
                                start=(k == 0), stop=(k == KF - 1 and not has_bout),
                                skip_group_check=True,
                            )
                        if has_bout:
                            nc.tensor.matmul(
                                out=ps[:rows, :],
                                lhsT=ones_sb[:, 0:rows], rhs=bout_sb[:, nsl],
                                start=False, stop=True, skip_group_check=True,
                            )
                        lo = lsb.tile([128, 500], F32, tag="lo")
                        if q % 2 == 0:
                            nc.vector.tensor_copy(out=lo[:rows, :], in_=ps[:rows, :])
                        else:
                            nc.scalar.copy(out=lo[:rows, :], in_=ps[:rows, :])
                        esc = lsb.tile([128, 500], BF16, tag="esc")
                        nc.scalar.activation(
                            esc[:rows, :], lo[:rows, :], AF.Exp,
                            accum_out=qsum[:rows, q : q + 1],
                        )
                        nc.sync.dma_start(
                            out=logits_out.ap()[j * 128 : j * 128 + rows, nsl],
                            in_=lo[:rows, :],
                        )
                    nc.vector.tensor_reduce(
                        out=sumexp_sb[:rows, j : j + 1], in_=qsum[:rows, :],
                        axis=mybir.AxisListType.X, op=OP.add,
                    )

                # encoder
                for t in range(T):
                    gru_step.t = t
                    gru_step(
                        xgT_dram.ap(), whhT_e_sb,
                        h_in=lambda k: h_bf[:, k * B : (k + 1) * B],
                        h_out=h_bf[:].rearrange("p (a b) -> p a b", a=KH),
                    )
                # decoder + interleaved output GEMM
                for t in range(T - 1):
                    gru_step.t = t
                    if t == 0:
                        h_in = lambda k: h_bf[:, k * B : (k + 1) * B]
                    else:
                        h_in = lambda k, _t=t: dec_hT[:, k, (_t - 1) * B : _t * B]
                    gru_step(ygT_dram.ap(), whhT_d_sb, h_in=h_in,
                             h_out=dec_hT[:, :, t * B : (t + 1) * B])
                    if t % 2 == 1:
                        gemm_mtile(t // 2)
                gemm_mtile(NMT - 1)

                # sumexp epilogue: [128, 32] -> [4032]
                nc.sync.dma_start(
                    out=sumexp_out.ap()[0 : 31 * 128].rearrange("(j p) -> p j", p=128),
                    in_=sumexp_sb[:, 0:31],
                )
                nc.sync.dma_start(
                    out=sumexp_out.ap()[31 * 128 : NTOK_Y].rearrange("(a b) -> a b", b=1),
                    in_=sumexp_sb[0:64, 31:32],
                )
    nc.compile()
    return nc


def _prep(inputs):
    """Host-side input prep: casts, transposes, shards."""
    f = {k: np.asarray(v) for k, v in inputs.items()}
    x = np.ascontiguousarray(
        f["x"].astype(np.int32).reshape(-1).reshape(NTOK_X // 128, 128).T
    )  # [128, 32]: tile i, partition p -> token i*128+p
    y_full = f["y"].astype(np.int32)
    y_flat = np.concatenate([y_full[:-1].reshape(-1), np.zeros(B, np.int32)])
    y = np.ascontiguousarray(y_flat.reshape(NTOK_X // 128, 128).T)
    xlens = np.ascontiguousarray(np.tile(f["x_lens"].astype(np.float32), (128, 1)))
    emb_bf = f["emb"].astype(ml_dtypes.bfloat16)
    bias_enc = (f["enc_bih"] + f["enc_bhh"]).astype(np.float32).copy()
    bias_enc[2 * H :] = f["enc_bih"][2 * H :]
    bias_dec = (f["dec_bih"] + f["dec_bhh"]).astype(np.float32).copy()
    bias_dec[2 * H :] = f["dec_bih"][2 * H :]
    # general-path limitation: bhh_n must be zero (exactness of n-gate). It is
    # zero in setup_inputs; fall back would need one extra per-step op.
    assert np.all(f["enc_bhh"][2 * H :] == 0) and np.all(f["dec_bhh"][2 * H :] == 0)

    common = {
        "xy_ids": np.concatenate([x, y], axis=1), "xlens": xlens, "emb_bf": emb_bf,
        "wihT_enc": np.ascontiguousarray(f["enc_Wih"].T).astype(ml_dtypes.bfloat16),
        "whhT_enc": np.ascontiguousarray(f["enc_Whh"].T).astype(ml_dtypes.bfloat16),
        "wihT_dec": np.ascontiguousarray(f["dec_Wih"].T).astype(ml_dtypes.bfloat16),
        "whhT_dec": np.ascontiguousarray(f["dec_Whh"].T).astype(ml_dtypes.bfloat16),
        "bias_enc": bias_enc, "bias_dec": bias_dec,
    }
    woutT = np.ascontiguousarray(f["Wout"].T).astype(ml_dtypes.bfloat16)  # [768, 32000]
    bout = f["bout"].astype(np.float32)
    has_bout = bool(np.any(bout != 0))
    in_maps = []
    for c in range(NCORES):
        m = dict(common)
        m["woutT"] = np.ascontiguousarray(woutT[:, c * VS : (c + 1) * VS])
        m["bout"] = np.ascontiguousarray(bout[c * VS : (c + 1) * VS])
        in_maps.append(m)
    return in_maps, has_bout, f


def kernel(**inputs):
    in_maps, has_bout, f = _prep(inputs)
    key = ("v1", has_bout)
    if key not in _CACHE:
        _CACHE[key] = _build(has_bout)
    nc = _CACHE[key]
    res = run_bass_kernel_spmd(nc, in_maps, list(range(NCORES)))

    logits = np.concatenate(
        [np.asarray(res.results[c]["logits_out"]) for c in range(NCORES)], axis=1
    )  # [4032, 32000] f32
    sumexp = np.stack(
        [np.asarray(res.results[c]["sumexp_out"]) for c in range(NCORES)], axis=0
    ).sum(axis=0)  # [4032]

    dec_out = np.concatenate(
        [np.zeros((1, B, V), np.float32), logits.reshape(T - 1, B, V)], axis=0
    )
    tgt = np.asarray(f["y"])[1:].reshape(-1).astype(np.int64)
    logz = np.log(sumexp)
    nll = logz - logits[np.arange(NTOK_Y), tgt]
    mask = (tgt != 0).astype(np.float32)
    loss = np.float32((nll * mask).sum() / max(mask.sum(), 1.0))
    return dec_out, loss


# revision 20
# speedup vs baseline: 1.6846x; 1.6846x over previous
"""Trainium2 Bass kernel for nn_AutoEncoder (GRU seq2seq + vocab projection + CE loss).

Sharding: vocab-sharded tensor parallelism for the dominant [4032,768]@[768,32000]
GEMM (core c owns Wout rows [c*4000,(c+1)*4000)); the latency-bound GRUs are
replicated on all 8 cores so no per-step collectives are needed.

Device program (per core, identical NEFF):
  - indirect-DMA embedding gathers + PE transposes -> exT/eyT (E on partitions)
  - input-gate GEMMs xgT/ygT staged to DRAM bf16 (T layout); z-gate carries
    bias + packed-seq mask (+30 where t >= len_b -> z saturates to exactly 1)
  - GRUs run in T layout (hidden on partitions): weights-stationary matmuls,
    n/r/z dispatch order, split tanh ops, tanh-only gate math
    (sigmoid(x) = 0.5(1+tanh(x/2))) so tanh+exp share one ACT table set;
    fp32 state s = 2h avoids bf16 drift and saves one op per step
  - the PE queue is in-order, so the big output GEMM is fed in half-m-tile
    chunks after every decoder step (fills GRU chain gaps without blocking
    the next step); yg input-gate units interleave into encoder steps
  - exp+rowsum fused via ACT accum_out during GEMM evacuation
Host: concatenates 8 logit shards, prepends the zero row, combines rowsum-exp
shards, and does the O(T*B) cross-entropy scalar math.
"""

import sys

sys.path.insert(0, "/opt/trn_rl_repo")

import numpy as np
import ml_dtypes

import concourse.bass as bass
import concourse.bacc as bacc
import concourse.mybir as mybir
import concourse.tile as tile
from concourse.masks import make_identity
from concourse.bass_utils import run_bass_kernel_spmd

BF16 = mybir.dt.bfloat16
F32 = mybir.dt.float32
I32 = mybir.dt.int32
AF = mybir.ActivationFunctionType
OP = mybir.AluOpType

V, E, H, T, B = 32000, 256, 512, 64, 64
H3 = 3 * H
NCORES = 8
VS = V // NCORES                # 4000
NTOK_X = T * B                  # 4096
NTOK_Y = (T - 1) * B            # 4032
KH = H // 128                   # 4
KE = E // 128                   # 2
MG = H3 // 128                  # 12
KF = (H + E) // 128             # 6
NMT = (NTOK_Y + 127) // 128     # 32 (last m-tile has 64 rows)
CH = 512                        # input-gate GEMM token chunk
MASK_BIG = 30.0

_CACHE = {}
PHASE_MARKS = []


def _build(has_bout):
    PHASE_MARKS.clear()
    nc = bacc.Bacc()
    mark = lambda s: PHASE_MARKS.append((s, nc.next_id()))

    xy_ids = nc.declare_dram_parameter("xy_ids", [128, 2 * (NTOK_X // 128)], I32, isOutput=False)
    xlens = nc.declare_dram_parameter("xlens", [128, B], F32, isOutput=False)
    emb_d = nc.declare_dram_parameter("emb_bf", [V, E], BF16, isOutput=False)
    wihT_e = nc.declare_dram_parameter("wihT_enc", [E, H3], BF16, isOutput=False)
    whhT_e = nc.declare_dram_parameter("whhT_enc", [H, H3], BF16, isOutput=False)
    wihT_d = nc.declare_dram_parameter("wihT_dec", [E, H3], BF16, isOutput=False)
    whhT_d = nc.declare_dram_parameter("whhT_dec", [H, H3], BF16, isOutput=False)
    woutT_d = nc.declare_dram_parameter("woutT", [KF * 128, VS], BF16, isOutput=False)
    bias_e = nc.declare_dram_parameter("bias_enc", [H3], F32, isOutput=False)
    bias_dd = nc.declare_dram_parameter("bias_dec", [H3], F32, isOutput=False)
    bout_d = nc.declare_dram_parameter("bout", [VS], F32, isOutput=False)
    logits_out = nc.declare_dram_parameter("logits_out", [NTOK_Y, VS], F32, isOutput=True)
    sumexp_out = nc.declare_dram_parameter("sumexp_out", [NTOK_Y], F32, isOutput=True)

    xgT_dram = nc.dram_tensor("xgT_stage", [128, MG, NTOK_X], BF16)
    ygT_dram = nc.dram_tensor("ygT_stage", [128, MG, NTOK_X], BF16)

    with tile.TileContext(nc) as tc:
        with (
            tc.tile_pool(name="persist", bufs=1) as pp,
            tc.tile_pool(name="state", bufs=1) as sp,
        ):
            ident = pp.tile([128, 128], BF16)
            make_identity(nc, ident[:])

            xy_sb = pp.tile([128, 2 * (NTOK_X // 128)], I32, tag="xyids")
            nc.gpsimd.dma_start(out=xy_sb[:], in_=xy_ids.ap())
            lens128 = pp.tile([128, B], F32, tag="lens128")
            nc.sync.dma_start(out=lens128[:], in_=xlens.ap())

            wihT_e_sb = pp.tile([128, KE, H3], BF16, tag="wihe")
            nc.sync.dma_start(out=wihT_e_sb[:], in_=wihT_e.ap().rearrange("(k p) m -> p k m", p=128))
            wihT_d_sb = pp.tile([128, KE, H3], BF16, tag="wihd")
            nc.sync.dma_start(out=wihT_d_sb[:], in_=wihT_d.ap().rearrange("(k p) m -> p k m", p=128))
            whhT_e_sb = pp.tile([128, KH, H3], BF16, tag="whhe")
            nc.sync.dma_start(out=whhT_e_sb[:], in_=whhT_e.ap().rearrange("(k p) m -> p k m", p=128))
            whhT_d_sb = pp.tile([128, KH, H3], BF16, tag="whhd")
            nc.sync.dma_start(out=whhT_d_sb[:], in_=whhT_d.ap().rearrange("(k p) m -> p k m", p=128))
            woutT_sb = pp.tile([128, KF, VS], BF16, tag="wout")
            nc.sync.dma_start(out=woutT_sb[:], in_=woutT_d.ap().rearrange("(k p) n -> p k n", p=128))
            bias_e_sb = pp.tile([128, MG], F32, tag="biase")
            nc.sync.dma_start(out=bias_e_sb[:], in_=bias_e.ap().rearrange("(m p) -> p m", p=128))
            bias_d_sb = pp.tile([128, MG], F32, tag="biasd")
            nc.sync.dma_start(out=bias_d_sb[:], in_=bias_dd.ap().rearrange("(m p) -> p m", p=128))
            if has_bout:
                bout_f_sb = pp.tile([1, VS], F32, tag="boutf")
                nc.sync.dma_start(out=bout_f_sb[:], in_=bout_d.ap().rearrange("(a b) -> a b", a=1))
                bout_sb = pp.tile([1, VS], BF16, tag="bout")
                nc.vector.tensor_copy(out=bout_sb[:], in_=bout_f_sb[:])
                ones_sb = pp.tile([1, 128], BF16, tag="ones")
                nc.vector.memset(ones_sb[:], 1.0)

            # mask30[p, t*64+b] = 30.0 where t >= len_b (identical rows)
            mask_tb = pp.tile([128, NTOK_X], BF16, tag="mask")
            for t in range(T):
                nc.vector.tensor_scalar(
                    out=mask_tb[:, t * B : (t + 1) * B], in0=lens128[:],
                    scalar1=float(t), scalar2=MASK_BIG,
                    op0=OP.is_le, op1=OP.mult,
                )

            # GRU state: s_f32 = 2*h; h_bf = bf16 h (matmul operand)
            s_f32 = sp.tile([128, KH * B], F32, tag="sf32")
            h_bf = sp.tile([128, KH * B], BF16, tag="hbf")
            nc.vector.memset(s_f32[:], 0.0)
            nc.vector.memset(h_bf[:], 0.0)
            dec_hT = sp.tile([128, KH, NTOK_Y], BF16, tag="dechT")
            eyT_sb = sp.tile([128, KE, NTOK_X], BF16, tag="eyT")
            sumexp2_sb = sp.tile([128, NMT, 2], F32, tag="sumexp2")
            sumexp_sb = sp.tile([128, NMT], F32, tag="sumexp")

            with (
                tc.tile_pool(name="exT_pool", bufs=1) as xp,
                tc.tile_pool(name="xgpsum", bufs=2, space="PSUM") as xgp,
                tc.tile_pool(name="xgsb", bufs=3) as xsb,
                tc.tile_pool(name="grurz", bufs=2, space="PSUM") as gpp_rz,
                tc.tile_pool(name="grun", bufs=1, space="PSUM") as gpp_n,
                tc.tile_pool(name="gempsum", bufs=2, space="PSUM") as lpp,
                tc.tile_pool(name="xgslice", bufs=4) as xsl,
                tc.tile_pool(name="gates", bufs=2) as gb,
                tc.tile_pool(name="lsb", bufs=2) as lsb,
                tc.tile_pool(name="escp", bufs=1) as escp,
            ):
                exT_sb = xp.tile([128, KE, NTOK_X], BF16, tag="exT")

                with (
                    tc.tile_pool(name="gath", bufs=8) as gp,
                    tc.tile_pool(name="tpsum", bufs=1, space="PSUM") as tpp,
                ):
                    def gather_transpose(ids_sb, dst):
                        for i in range(NTOK_X // 128):
                            g = gp.tile([128, E], BF16, tag="gtile")
                            nc.gpsimd.indirect_dma_start(
                                out=g[:],
                                out_offset=None,
                                in_=emb_d.ap(),
                                in_offset=bass.IndirectOffsetOnAxis(ap=ids_sb[:, i : i + 1], axis=0),
                            )
                            for k in range(KE):
                                ps = tpp.tile([128, 128], BF16, tag="tp")
                                nc.tensor.transpose(
                                    out=ps[:], in_=g[:, k * 128 : (k + 1) * 128], identity=ident[:]
                                )
                                nc.vector.tensor_copy(out=dst[:, k, i * 128 : (i + 1) * 128], in_=ps[:])

                    mark("gathers")
                    gather_transpose(xy_sb, exT_sb)
                    gather_transpose(xy_sb[:, NTOK_X // 128 :], eyT_sb)

                def ig_unit(wih_sb, src_T, dst_dram, bias_sb, masked, c, j):
                    ps = xgp.tile([128, CH], F32, tag="xgps")
                    for k in range(KE):
                        nc.tensor.matmul(
                            out=ps[:],
                            lhsT=wih_sb[:, k, j * 128 : (j + 1) * 128],
                            rhs=src_T[:, k, c * CH : (c + 1) * CH],
                            start=(k == 0), stop=(k == KE - 1),
                        )
                    ev = xsb.tile([128, CH], BF16, tag="xgev")
                    if masked and 4 <= j < 8:
                        nc.vector.scalar_tensor_tensor(
                            out=ev[:], in0=ps[:],
                            scalar=bias_sb[:, j : j + 1],
                            in1=mask_tb[:, c * CH : (c + 1) * CH],
                            op0=OP.add, op1=OP.add,
                        )
                    else:
                        nc.vector.tensor_scalar(
                            out=ev[:], in0=ps[:],
                            scalar1=bias_sb[:, j : j + 1], scalar2=None,
                            op0=OP.add,
                        )
                    nc.sync.dma_start(out=dst_dram[:, j, c * CH : (c + 1) * CH], in_=ev[:])

                mark("xg_gemm")
                for c in range(NTOK_X // CH):
                    for j in range(MG):
                        ig_unit(wihT_e_sb, exT_sb, xgT_dram.ap(), bias_e_sb, True, c, j)
                yg_units = [
                    (wihT_d_sb, eyT_sb, ygT_dram.ap(), bias_d_sb, False, c, j)
                    for c in range(NTOK_X // CH) for j in range(MG)
                ]

                RZ, NN = 8 * B, 4 * B  # 512, 256

                def gru_step(xg_dram, whh_sb, t, h_in, h_out):
                    xg = xsl.tile([128, MG, B], BF16, tag="xgsl")
                    nc.sync.dma_start(out=xg[:], in_=xg_dram[:, :, bass.ts(t, B)])
                    ps_rz = gpp_rz.tile([128, RZ], F32, tag="gpsrz")
                    ps_n = gpp_n.tile([128, NN], F32, tag="gpsn")
                    # xg preload for r,z - independent of h, dispatches early
                    nc.tensor.matmul(
                        out=ps_rz[:], lhsT=ident[:],
                        rhs=xg[:, 0:8, :].rearrange("p a b -> p (a b)"),
                        start=True, stop=False, skip_group_check=True,
                    )
                    # n-tiles first, then r, then z (earliest tr availability)
                    for j in (8, 9, 10, 11, 0, 1, 2, 3, 4, 5, 6, 7):
                        out_ap = (
                            ps_n[:, (j - 8) * B : (j - 7) * B] if j >= 8
                            else ps_rz[:, j * B : (j + 1) * B]
                        )
                        for k in range(KH):
                            nc.tensor.matmul(
                                out=out_ap,
                                lhsT=whh_sb[:, k, j * 128 : (j + 1) * 128],
                                rhs=h_in(k),
                                start=(j == 8 and k == 0),
                                stop=(k == KH - 1),
                                skip_group_check=True,
                            )
                    tr = gb.tile([128, NN], BF16, tag="tr")
                    nc.scalar.activation(tr[:], ps_rz[:, 0:NN], AF.Tanh, scale=0.5)
                    tz = gb.tile([128, NN], BF16, tag="tz")
                    nc.scalar.activation(tz[:], ps_rz[:, NN:RZ], AF.Tanh, scale=0.5)
                    pp_t = gb.tile([128, NN], BF16, tag="ppt")
                    nc.vector.scalar_tensor_tensor(
                        out=pp_t[:], in0=tr[:], scalar=1.0, in1=ps_n[:],
                        op0=OP.add, op1=OP.mult,
                    )
                    an_t = gb.tile([128, NN], BF16, tag="ant")
                    nc.vector.scalar_tensor_tensor(
                        out=an_t[:], in0=pp_t[:], scalar=0.5,
                        in1=xg[:, 8:12, :].rearrange("p a b -> p (a b)"),
                        op0=OP.mult, op1=OP.add,
                    )
                    n_t = gb.tile([128, NN], BF16, tag="nt")
                    nc.scalar.activation(n_t[:], an_t[:], AF.Tanh)
                    v_t = gb.tile([128, NN], F32, tag="vt")
                    nc.vector.scalar_tensor_tensor(
                        out=v_t[:], in0=tz[:], scalar=1.0, in1=s_f32[:],
                        op0=OP.add, op1=OP.mult,
                    )
                    u_t = gb.tile([128, NN], BF16, tag="ut")
                    nc.vector.scalar_tensor_tensor(
                        out=u_t[:], in0=tz[:], scalar=1.0, in1=n_t[:],
                        op0=OP.subtract, op1=OP.mult,
                    )
                    # s' = 2h' = 0.5*v - u   (freeze: tz=1 -> v=2s, u=0, s'=s exactly)
                    nc.vector.scalar_tensor_tensor(
                        out=s_f32[:], in0=v_t[:], scalar=0.5, in1=u_t[:],
                        op0=OP.mult, op1=OP.subtract,
                    )
                    nc.vector.tensor_scalar(
                        out=h_out, in0=s_f32[:].rearrange("p (a b) -> p a b", a=KH),
                        scalar1=0.5, scalar2=None, op0=OP.mult,
                    )

                def gemm_half(j, rows, half):
                    lo = lsb.tile([128, 2000], F32, tag="lo")
                    for qq in range(4):
                        q = half * 4 + qq
                        nsl = slice(q * 500, (q + 1) * 500)
                        ps = lpp.tile([128, 500], F32, tag="lps")
                        for k in range(KF):
                            lhsT = (
                                dec_hT[:, k, j * 128 : j * 128 + rows]
                                if k < KH
                                else eyT_sb[:, k - KH, j * 128 : j * 128 + rows]
                            )
                            nc.tensor.matmul(
                                out=ps[:rows, :], lhsT=lhsT, rhs=woutT_sb[:, k, nsl],
                                start=(k == 0), stop=(k == KF - 1 and not has_bout),
                                skip_group_check=True,
                            )
                        if has_bout:
                            nc.tensor.matmul(
                                out=ps[:rows, :],
                                lhsT=ones_sb[:, 0:rows], rhs=bout_sb[:, nsl],
                                start=False, stop=True, skip_group_check=True,
                            )
                        if qq == 3:
                            nc.scalar.copy(out=lo[:rows, qq * 500 : (qq + 1) * 500], in_=ps[:rows, :])
                        else:
                            nc.vector.tensor_copy(
                                out=lo[:rows, qq * 500 : (qq + 1) * 500], in_=ps[:rows, :]
                            )
                    esc = escp.tile([128, 2000], BF16, tag="esc")
                    nc.scalar.activation(
                        esc[:rows, :], lo[:rows, :], AF.Exp,
                        accum_out=sumexp2_sb[:rows, j, half : half + 1],
                    )
                    nc.sync.dma_start(
                        out=logits_out.ap()[
                            j * 128 : j * 128 + rows, half * 2000 : (half + 1) * 2000
                        ],
                        in_=lo[:rows, :],
                    )

                # encoder, yg input-gate units interleaved (in-order PE queue:
                # big pre-dispatched GEMMs would block the next GRU step)
                mark("enc")
                yi = 0
                for t in range(T):
                    gru_step(
                        xgT_dram.ap(), whhT_e_sb, t,
                        h_in=lambda k: h_bf[:, k * B : (k + 1) * B],
                        h_out=h_bf[:].rearrange("p (a b) -> p a b", a=KH),
                    )
                    for _ in range(2):
                        if yi < len(yg_units):
                            ig_unit(*yg_units[yi]); yi += 1
                while yi < len(yg_units):
                    ig_unit(*yg_units[yi]); yi += 1

                # decoder with output-GEMM halves after every step
                mark("dec")
                pending = []
                for t in range(T - 1):
                    if t == 0:
                        h_in = lambda k: h_bf[:, k * B : (k + 1) * B]
                    else:
                        h_in = lambda k, _t=t: dec_hT[:, k, (_t - 1) * B : _t * B]
                    gru_step(ygT_dram.ap(), whhT_d_sb, t, h_in=h_in,
                             h_out=dec_hT[:, :, t * B : (t + 1) * B])
                    if t % 2 == 1:
                        pending.append((t // 2, 128, 0))
                        pending.append((t // 2, 128, 1))
                    if pending:
                        gemm_half(*pending.pop(0))
                pending.append((NMT - 1, 64, 0))
                pending.append((NMT - 1, 64, 1))
                mark("tail")
                for args in pending:
                    gemm_half(*args)

                # sumexp: combine halves, then DMA [128,32] -> [4032]
                nc.vector.tensor_reduce(
                    out=sumexp_sb[:, :], in_=sumexp2_sb[:, :, :],
                    axis=mybir.AxisListType.X, op=OP.add,
                )
                nc.sync.dma_start(
                    out=sumexp_out.ap()[0 : 31 * 128].rearrange("(j p) -> p j", p=128),
                    in_=sumexp_sb[:, 0:31],
                )
                nc.sync.dma_start(
                    out=sumexp_out.ap()[31 * 128 : NTOK_Y].rearrange("(a b) -> a b", b=1),
                    in_=sumexp_sb[0:64, 31:32],
                )
    nc.compile()
    return nc


def _prep(inputs):
    f = {k: np.asarray(v) for k, v in inputs.items()}
    x = np.ascontiguousarray(
        f["x"].astype(np.int32).reshape(-1).reshape(NTOK_X // 128, 128).T
    )
    y_full = f["y"].astype(np.int32)
    y_flat = np.concatenate([y_full[:-1].reshape(-1), np.zeros(B, np.int32)])
    y = np.ascontiguousarray(y_flat.reshape(NTOK_X // 128, 128).T)
    xlens = np.ascontiguousarray(np.tile(f["x_lens"].astype(np.float32), (128, 1)))
    emb_bf = f["emb"].astype(ml_dtypes.bfloat16)
    bias_enc = (f["enc_bih"] + f["enc_bhh"]).astype(np.float32).copy()
    bias_enc[2 * H :] = f["enc_bih"][2 * H :]
    bias_dec = (f["dec_bih"] + f["dec_bhh"]).astype(np.float32).copy()
    bias_dec[2 * H :] = f["dec_bih"][2 * H :]
    # n-gate exactness requires bhh_n == 0 (true for this model's init)
    assert np.all(f["enc_bhh"][2 * H :] == 0) and np.all(f["dec_bhh"][2 * H :] == 0)

    common = {
        "xy_ids": np.concatenate([x, y], axis=1), "xlens": xlens, "emb_bf": emb_bf,
        "wihT_enc": np.ascontiguousarray(f["enc_Wih"].T).astype(ml_dtypes.bfloat16),
        "whhT_enc": np.ascontiguousarray(f["enc_Whh"].T).astype(ml_dtypes.bfloat16),
        "wihT_dec": np.ascontiguousarray(f["dec_Wih"].T).astype(ml_dtypes.bfloat16),
        "whhT_dec": np.ascontiguousarray(f["dec_Whh"].T).astype(ml_dtypes.bfloat16),
        "bias_enc": bias_enc, "bias_dec": bias_dec,
    }
    woutT = np.ascontiguousarray(f["Wout"].T).astype(ml_dtypes.bfloat16)
    bout = f["bout"].astype(np.float32)
    has_bout = bool(np.any(bout != 0))
    in_maps = []
    for c in range(NCORES):
        m = dict(common)
        m["woutT"] = np.ascontiguousarray(woutT[:, c * VS : (c + 1) * VS])
        m["bout"] = np.ascontiguousarray(bout[c * VS : (c + 1) * VS])
        in_maps.append(m)
    return in_maps, has_bout, f


def kernel(**inputs):
    in_maps, has_bout, f = _prep(inputs)
    key = ("v2", has_bout)
    if key not in _CACHE:
        _CACHE[key] = _build(has_bout)
    nc = _CACHE[key]
    res = run_bass_kernel_spmd(nc, in_maps, list(range(NCORES)))

    logits = np.concatenate(
        [np.asarray(res.results[c]["logits_out"]) for c in range(NCORES)], axis=1
    )
    sumexp = np.stack(
        [np.asarray(res.results[c]["sumexp_out"]) for c in range(NCORES)], axis=0
    ).sum(axis=0)

    dec_out = np.concatenate(
        [np.zeros((1, B, V), np.float32), logits.reshape(T - 1, B, V)], axis=0
    )
    tgt = np.asarray(f["y"])[1:].reshape(-1).astype(np.int64)
    logz = np.log(sumexp)
    nll = logz - logits[np.arange(NTOK_Y), tgt]
    mask = (tgt != 0).astype(np.float32)
    loss = np.float32((nll * mask).sum() / max(mask.sum(), 1.0))
    return dec_out, loss


# revision 26
# speedup vs baseline: 37122.5002x; 22036.6292x over previous
"""Trainium2 Bass kernel for nn_AutoEncoder (GRU seq2seq + vocab projection + CE loss).

Sharding: vocab-sharded tensor parallelism for the dominant [4032,768]@[768,32000]
GEMM (core c owns Wout rows [c*4000,(c+1)*4000)); the latency-bound GRUs are
replicated on all 8 cores so no per-step collectives are needed.

Device program (per core, identical NEFF):
  - indirect-DMA embedding gathers + PE transposes -> exT/eyT (E on partitions)
  - input-gate GEMMs xgT/ygT staged to DRAM bf16 (T layout); z-gate carries
    bias + packed-seq mask (+30 where t >= len_b -> z saturates to exactly 1)
  - GRUs run in T layout (hidden on partitions): weights-stationary matmuls,
    n/r/z dispatch order, split tanh ops, tanh-only gate math
    (sigmoid(x) = 0.5(1+tanh(x/2))) so tanh+exp share one ACT table set;
    fp32 state s = 2h avoids bf16 drift and saves one op per step
  - the PE queue is in-order, so the big output GEMM is fed in half-m-tile
    chunks after every decoder step (fills GRU chain gaps without blocking
    the next step); yg input-gate units interleave into encoder steps
  - exp+rowsum fused via ACT accum_out during GEMM evacuation
Host: concatenates 8 logit shards, prepends the zero row, combines rowsum-exp
shards, and does the O(T*B) cross-entropy scalar math.
"""

import sys

sys.path.insert(0, "/opt/trn_rl_repo")

import numpy as np
import ml_dtypes

import concourse.bass as bass
import concourse.bacc as bacc
import concourse.mybir as mybir
import concourse.tile as tile
from concourse.masks import make_identity
from concourse.bass_utils import run_bass_kernel_spmd

BF16 = mybir.dt.bfloat16
F32 = mybir.dt.float32
I32 = mybir.dt.int32
AF = mybir.ActivationFunctionType
OP = mybir.AluOpType

V, E, H, T, B = 32000, 256, 512, 64, 64
H3 = 3 * H
NCORES = 8
VS = V // NCORES                # 4000
NTOK_X = T * B                  # 4096
NTOK_Y = (T - 1) * B            # 4032
KH = H // 128                   # 4
KE = E // 128                   # 2
MG = H3 // 128                  # 12
KF = (H + E) // 128             # 6
NMT = (NTOK_Y + 127) // 128     # 32 (last m-tile has 64 rows)
CH = 512                        # input-gate GEMM token chunk
MASK_BIG = 30.0

_CACHE = {}
PHASE_MARKS = []


def _build(has_bout):
    PHASE_MARKS.clear()
    nc = bacc.Bacc()
    mark = lambda s: PHASE_MARKS.append((s, nc.next_id()))

    xy_ids = nc.declare_dram_parameter("xy_ids", [128, 2 * (NTOK_X // 128)], I32, isOutput=False)
    xlens = nc.declare_dram_parameter("xlens", [128, B], F32, isOutput=False)
    emb_d = nc.declare_dram_parameter("emb_bf", [V, E], BF16, isOutput=False)
    wihT_e = nc.declare_dram_parameter("wihT_enc", [E, H3], BF16, isOutput=False)
    whhT_e = nc.declare_dram_parameter("whhT_enc", [H, H3], BF16, isOutput=False)
    wihT_d = nc.declare_dram_parameter("wihT_dec", [E, H3], BF16, isOutput=False)
    whhT_d = nc.declare_dram_parameter("whhT_dec", [H, H3], BF16, isOutput=False)
    woutT_d = nc.declare_dram_parameter("woutT", [KF * 128, VS], BF16, isOutput=False)
    bias_e = nc.declare_dram_parameter("bias_enc", [H3], F32, isOutput=False)
    bias_dd = nc.declare_dram_parameter("bias_dec", [H3], F32, isOutput=False)
    bout_d = nc.declare_dram_parameter("bout", [VS], F32, isOutput=False)
    logits_out = nc.declare_dram_parameter("logits_out", [NTOK_Y, VS], F32, isOutput=True)
    sumexp_out = nc.declare_dram_parameter("sumexp_out", [NTOK_Y], F32, isOutput=True)

    xgT_dram = nc.dram_tensor("xgT_stage", [128, MG, NTOK_X], BF16)
    ygT_dram = nc.dram_tensor("ygT_stage", [128, MG, NTOK_X], BF16)

    with tile.TileContext(nc) as tc:
        with (
            tc.tile_pool(name="persist", bufs=1) as pp,
            tc.tile_pool(name="state", bufs=1) as sp,
        ):
            ident = pp.tile([128, 128], BF16)
            make_identity(nc, ident[:])

            xy_sb = pp.tile([128, 2 * (NTOK_X // 128)], I32, tag="xyids")
            nc.gpsimd.dma_start(out=xy_sb[:], in_=xy_ids.ap())
            lens128 = pp.tile([128, B], F32, tag="lens128")
            nc.sync.dma_start(out=lens128[:], in_=xlens.ap())

            wihT_e_sb = pp.tile([128, KE, H3], BF16, tag="wihe")
            nc.sync.dma_start(out=wihT_e_sb[:], in_=wihT_e.ap().rearrange("(k p) m -> p k m", p=128))
            wihT_d_sb = pp.tile([128, KE, H3], BF16, tag="wihd")
            nc.sync.dma_start(out=wihT_d_sb[:], in_=wihT_d.ap().rearrange("(k p) m -> p k m", p=128))
            whhT_e_sb = pp.tile([128, KH, H3], BF16, tag="whhe")
            nc.sync.dma_start(out=whhT_e_sb[:], in_=whhT_e.ap().rearrange("(k p) m -> p k m", p=128))
            whhT_d_sb = pp.tile([128, KH, H3], BF16, tag="whhd")
            nc.sync.dma_start(out=whhT_d_sb[:], in_=whhT_d.ap().rearrange("(k p) m -> p k m", p=128))
            woutT_sb = pp.tile([128, KF, VS], BF16, tag="wout")
            nc.sync.dma_start(out=woutT_sb[:], in_=woutT_d.ap().rearrange("(k p) n -> p k n", p=128))
            bias_e_sb = pp.tile([128, MG], F32, tag="biase")
            nc.sync.dma_start(out=bias_e_sb[:], in_=bias_e.ap().rearrange("(m p) -> p m", p=128))
            bias_d_sb = pp.tile([128, MG], F32, tag="biasd")
            nc.sync.dma_start(out=bias_d_sb[:], in_=bias_dd.ap().rearrange("(m p) -> p m", p=128))
            if has_bout:
                bout_f_sb = pp.tile([1, VS], F32, tag="boutf")
                nc.sync.dma_start(out=bout_f_sb[:], in_=bout_d.ap().rearrange("(a b) -> a b", a=1))
                bout_sb = pp.tile([1, VS], BF16, tag="bout")
                nc.vector.tensor_copy(out=bout_sb[:], in_=bout_f_sb[:])
                ones_sb = pp.tile([1, 128], BF16, tag="ones")
                nc.vector.memset(ones_sb[:], 1.0)

            # mask30[p, t*64+b] = 30.0 where t >= len_b (identical rows)
            mask_tb = pp.tile([128, NTOK_X], BF16, tag="mask")
            for t in range(T):
                nc.vector.tensor_scalar(
                    out=mask_tb[:, t * B : (t + 1) * B], in0=lens128[:],
                    scalar1=float(t), scalar2=MASK_BIG,
                    op0=OP.is_le, op1=OP.mult,
                )

            # GRU state: s_f32 = 2*h; h_bf = bf16 h (matmul operand)
            s_f32 = sp.tile([128, KH * B], F32, tag="sf32")
            h_bf = sp.tile([128, KH * B], BF16, tag="hbf")
            nc.vector.memset(s_f32[:], 0.0)
            nc.vector.memset(h_bf[:], 0.0)
            dec_hT = sp.tile([128, KH, NTOK_Y], BF16, tag="dechT")
            eyT_sb = sp.tile([128, KE, NTOK_X], BF16, tag="eyT")
            sumexp2_sb = sp.tile([128, NMT, 2], F32, tag="sumexp2")
            sumexp_sb = sp.tile([128, NMT], F32, tag="sumexp")

            with (
                tc.tile_pool(name="exT_pool", bufs=1) as xp,
                tc.tile_pool(name="xgpsum", bufs=2, space="PSUM") as xgp,
                tc.tile_pool(name="xgsb", bufs=3) as xsb,
                tc.tile_pool(name="grurz", bufs=2, space="PSUM") as gpp_rz,
                tc.tile_pool(name="grun", bufs=1, space="PSUM") as gpp_n,
                tc.tile_pool(name="gempsum", bufs=2, space="PSUM") as lpp,
                tc.tile_pool(name="xgslice", bufs=6) as xsl,
                tc.tile_pool(name="gates", bufs=3) as gb,
                tc.tile_pool(name="lsb", bufs=2) as lsb,
                tc.tile_pool(name="escp", bufs=1) as escp,
                tc.tile_pool(name="gath", bufs=8) as gp,
                tc.tile_pool(name="tpsum", bufs=1, space="PSUM") as tpp,
            ):
                exT_sb = xp.tile([128, KE, NTOK_X], BF16, tag="exT")

                def gt_unit(ids_sb, dst, i):
                    g = gp.tile([128, E], BF16, tag="gtile")
                    nc.gpsimd.indirect_dma_start(
                        out=g[:],
                        out_offset=None,
                        in_=emb_d.ap(),
                        in_offset=bass.IndirectOffsetOnAxis(ap=ids_sb[:, i : i + 1], axis=0),
                    )
                    for k in range(KE):
                        ps = tpp.tile([128, 128], BF16, tag="tp")
                        nc.tensor.transpose(
                            out=ps[:], in_=g[:, k * 128 : (k + 1) * 128], identity=ident[:]
                        )
                        nc.scalar.copy(out=dst[:, k, i * 128 : (i + 1) * 128], in_=ps[:])

                mark("gathers")
                for i in range(NTOK_X // 128):
                    gt_unit(xy_sb, exT_sb, i)

                def ig_unit(wih_sb, src_T, dst_dram, bias_sb, masked, c, j):
                    ps = xgp.tile([128, CH], F32, tag="xgps")
                    for k in range(KE):
                        nc.tensor.matmul(
                            out=ps[:],
                            lhsT=wih_sb[:, k, j * 128 : (j + 1) * 128],
                            rhs=src_T[:, k, c * CH : (c + 1) * CH],
                            start=(k == 0), stop=(k == KE - 1),
                        )
                    ev = xsb.tile([128, CH], BF16, tag="xgev")
                    if masked and 4 <= j < 8:
                        nc.vector.scalar_tensor_tensor(
                            out=ev[:], in0=ps[:],
                            scalar=bias_sb[:, j : j + 1],
                            in1=mask_tb[:, c * CH : (c + 1) * CH],
                            op0=OP.add, op1=OP.add,
                        )
                    else:
                        # ACT evac keeps the DVE queue clear for the GRU chain
                        nc.scalar.add(out=ev[:], in_=ps[:], add=bias_sb[:, j : j + 1])
                    nc.sync.dma_start(out=dst_dram[:, j, c * CH : (c + 1) * CH], in_=ev[:])

                mark("xg_gemm")
                for c in range(NTOK_X // CH):
                    for j in range(MG):
                        ig_unit(wihT_e_sb, exT_sb, xgT_dram.ap(), bias_e_sb, True, c, j)
                # encoder-step fillers: y gathers/transposes first, then yg units
                fillers = [
                    (lambda _i=i: gt_unit(xy_sb[:, NTOK_X // 128 :], eyT_sb, _i))
                    for i in range(NTOK_X // 128)
                ] + [
                    (lambda _c=c, _j=j: ig_unit(
                        wihT_d_sb, eyT_sb, ygT_dram.ap(), bias_d_sb, False, _c, _j))
                    for c in range(NTOK_X // CH) for j in range(MG)
                ]

                RZ, NN = 8 * B, 4 * B  # 512, 256

                def gru_step(xg_dram, whh_sb, t, h_in, h_out):
                    xg = xsl.tile([128, MG, B], BF16, tag="xgsl")
                    nc.sync.dma_start(out=xg[:], in_=xg_dram[:, :, bass.ts(t, B)])
                    ps_rz = gpp_rz.tile([128, RZ], F32, tag="gpsrz")
                    ps_n = gpp_n.tile([128, NN], F32, tag="gpsn")
                    # xg preload for r,z - independent of h, dispatches early
                    nc.tensor.matmul(
                        out=ps_rz[:], lhsT=ident[:],
                        rhs=xg[:, 0:8, :].rearrange("p a b -> p (a b)"),
                        start=True, stop=False, skip_group_check=True,
                    )
                    # n-tiles first, then r, then z (earliest tr availability)
                    for j in (8, 9, 10, 11, 0, 1, 2, 3, 4, 5, 6, 7):
                        out_ap = (
                            ps_n[:, (j - 8) * B : (j - 7) * B] if j >= 8
                            else ps_rz[:, j * B : (j + 1) * B]
                        )
                        for k in range(KH):
                            nc.tensor.matmul(
                                out=out_ap,
                                lhsT=whh_sb[:, k, j * 128 : (j + 1) * 128],
                                rhs=h_in(k),
                                start=(j == 8 and k == 0),
                                stop=(k == KH - 1),
                                skip_group_check=True,
                            )
                    tr = gb.tile([128, NN], BF16, tag="tr")
                    nc.scalar.activation(tr[:], ps_rz[:, 0:NN], AF.Tanh, scale=0.5)
                    tz = gb.tile([128, NN], BF16, tag="tz")
                    nc.scalar.activation(tz[:], ps_rz[:, NN:RZ], AF.Tanh, scale=0.5)
                    pp_t = gb.tile([128, NN], BF16, tag="ppt")
                    nc.vector.scalar_tensor_tensor(
                        out=pp_t[:], in0=tr[:], scalar=1.0, in1=ps_n[:],
                        op0=OP.add, op1=OP.mult,
                    )
                    an_t = gb.tile([128, NN], BF16, tag="ant")
                    nc.vector.scalar_tensor_tensor(
                        out=an_t[:], in0=pp_t[:], scalar=0.5,
                        in1=xg[:, 8:12, :].rearrange("p a b -> p (a b)"),
                        op0=OP.mult, op1=OP.add,
                    )
                    n_t = gb.tile([128, NN], BF16, tag="nt")
                    nc.scalar.activation(n_t[:], an_t[:], AF.Tanh)
                    v_t = gb.tile([128, NN], F32, tag="vt")
                    nc.vector.scalar_tensor_tensor(
                        out=v_t[:], in0=tz[:], scalar=1.0, in1=s_f32[:],
                        op0=OP.add, op1=OP.mult,
                    )
                    u_t = gb.tile([128, NN], BF16, tag="ut")
                    nc.vector.scalar_tensor_tensor(
                        out=u_t[:], in0=tz[:], scalar=1.0, in1=n_t[:],
                        op0=OP.subtract, op1=OP.mult,
                    )
                    # s' = 2h' = 0.5*v - u   (freeze: tz=1 -> v=2s, u=0, s'=s exactly)
                    nc.vector.scalar_tensor_tensor(
                        out=s_f32[:], in0=v_t[:], scalar=0.5, in1=u_t[:],
                        op0=OP.mult, op1=OP.subtract,
                    )
                    nc.vector.tensor_scalar(
                        out=h_out, in0=s_f32[:].rearrange("p (a b) -> p a b", a=KH),
                        scalar1=0.5, scalar2=None, op0=OP.mult,
                    )

                def gemm_half(j, rows, half):
                    lo = lsb.tile([128, 2000], F32, tag="lo")
                    for qq in range(4):
                        q = half * 4 + qq
                        nsl = slice(q * 500, (q + 1) * 500)
                        ps = lpp.tile([128, 500], F32, tag="lps")
                        for k in range(KF):
                            lhsT = (
                                dec_hT[:, k, j * 128 : j * 128 + rows]
                                if k < KH
                                else eyT_sb[:, k - KH, j * 128 : j * 128 + rows]
                            )
                            nc.tensor.matmul(
                                out=ps[:rows, :], lhsT=lhsT, rhs=woutT_sb[:, k, nsl],
                                start=(k == 0), stop=(k == KF - 1 and not has_bout),
                                skip_group_check=True,
                            )
                        if has_bout:
                            nc.tensor.matmul(
                                out=ps[:rows, :],
                                lhsT=ones_sb[:, 0:rows], rhs=bout_sb[:, nsl],
                                start=False, stop=True, skip_group_check=True,
                            )
                        if qq == 3:
                            nc.scalar.copy(out=lo[:rows, qq * 500 : (qq + 1) * 500], in_=ps[:rows, :])
                        else:
                            nc.vector.tensor_copy(
                                out=lo[:rows, qq * 500 : (qq + 1) * 500], in_=ps[:rows, :]
                            )
                    esc = escp.tile([128, 2000], BF16, tag="esc")
                    nc.scalar.activation(
                        esc[:rows, :], lo[:rows, :], AF.Exp,
                        accum_out=sumexp2_sb[:rows, j, half : half + 1],
                    )
                    nc.sync.dma_start(
                        out=logits_out.ap()[
                            j * 128 : j * 128 + rows, half * 2000 : (half + 1) * 2000
                        ],
                        in_=lo[:rows, :],
                    )

                # encoder, yg input-gate units interleaved (in-order PE queue:
                # big pre-dispatched GEMMs would block the next GRU step)
                mark("enc")
                yi = 0
                for t in range(T):
                    gru_step(
                        xgT_dram.ap(), whhT_e_sb, t,
                        h_in=lambda k: h_bf[:, k * B : (k + 1) * B],
                        h_out=h_bf[:].rearrange("p (a b) -> p a b", a=KH),
                    )
                    for _ in range(2):
                        if yi < len(fillers):
                            fillers[yi](); yi += 1
                while yi < len(fillers):
                    fillers[yi](); yi += 1

                # decoder with output-GEMM halves after every step
                mark("dec")
                pending = []
                for t in range(T - 1):
                    if t == 0:
                        h_in = lambda k: h_bf[:, k * B : (k + 1) * B]
                    else:
                        h_in = lambda k, _t=t: dec_hT[:, k, (_t - 1) * B : _t * B]
                    gru_step(ygT_dram.ap(), whhT_d_sb, t, h_in=h_in,
                             h_out=dec_hT[:, :, t * B : (t + 1) * B])
                    if t % 2 == 1:
                        pending.append((t // 2, 128, 0))
                        pending.append((t // 2, 128, 1))
                    if pending:
                        gemm_half(*pending.pop(0))
                pending.append((NMT - 1, 64, 0))
                pending.append((NMT - 1, 64, 1))
                mark("tail")
                for args in pending:
                    gemm_half(*args)

                # sumexp: combine halves, then DMA [128,32] -> [4032]
                nc.vector.tensor_reduce(
                    out=sumexp_sb[:, :], in_=sumexp2_sb[:, :, :],
                    axis=mybir.AxisListType.X, op=OP.add,
                )
                nc.sync.dma_start(
                    out=sumexp_out.ap()[0 : 31 * 128].rearrange("(j p) -> p j", p=128),
                    in_=sumexp_sb[:, 0:31],
                )
                nc.sync.dma_start(
                    out=sumexp_out.ap()[31 * 128 : NTOK_Y].rearrange("(a b) -> a b", b=1),
                    in_=sumexp_sb[0:64, 31:32],
                )
    nc.compile()
    return nc


def _prep(inputs):
    f = {k: np.asarray(v) for k, v in inputs.items()}
    x = np.ascontiguousarray(
        f["x"].astype(np.int32).reshape(-1).reshape(NTOK_X // 128, 128).T
    )
    y_full = f["y"].astype(np.int32)
    y_flat = np.concatenate([y_full[:-1].reshape(-1), np.zeros(B, np.int32)])
    y = np.ascontiguousarray(y_flat.reshape(NTOK_X // 128, 128).T)
    xlens = np.ascontiguousarray(np.tile(f["x_lens"].astype(np.float32), (128, 1)))
    emb_bf = f["emb"].astype(ml_dtypes.bfloat16)
    bias_enc = (f["enc_bih"] + f["enc_bhh"]).astype(np.float32).copy()
    bias_enc[2 * H :] = f["enc_bih"][2 * H :]
    bias_dec = (f["dec_bih"] + f["dec_bhh"]).astype(np.float32).copy()
    bias_dec[2 * H :] = f["dec_bih"][2 * H :]
    # n-gate exactness requires bhh_n == 0 (true for this model's init)
    assert np.all(f["enc_bhh"][2 * H :] == 0) and np.all(f["dec_bhh"][2 * H :] == 0)

    common = {
        "xy_ids": np.concatenate([x, y], axis=1), "xlens": xlens, "emb_bf": emb_bf,
        "wihT_enc": np.ascontiguousarray(f["enc_Wih"].T).astype(ml_dtypes.bfloat16),
        "whhT_enc": np.ascontiguousarray(f["enc_Whh"].T).astype(ml_dtypes.bfloat16),
        "wihT_dec": np.ascontiguousarray(f["dec_Wih"].T).astype(ml_dtypes.bfloat16),
        "whhT_dec": np.ascontiguousarray(f["dec_Whh"].T).astype(ml_dtypes.bfloat16),
        "bias_enc": bias_enc, "bias_dec": bias_dec,
    }
    woutT = np.ascontiguousarray(f["Wout"].T).astype(ml_dtypes.bfloat16)
    bout = f["bout"].astype(np.float32)
    has_bout = bool(np.any(bout != 0))
    in_maps = []
    for c in range(NCORES):
        m = dict(common)
        m["woutT"] = np.ascontiguousarray(woutT[:, c * VS : (c + 1) * VS])
        m["bout"] = np.ascontiguousarray(bout[c * VS : (c + 1) * VS])
        in_maps.append(m)
    return in_maps, has_bout, f


def kernel(**inputs):
    in_maps, has_bout, f = _prep(inputs)
    key = ("v2", has_bout)
    if key not in _CACHE:
        _CACHE[key] = _build(has_bout)
    nc = _CACHE[key]
    res = run_bass_kernel_spmd(nc, in_maps, list(range(NCORES)))

    logits = np.concatenate(
        [np.asarray(res.results[c]["logits_out"]) for c in range(NCORES)], axis=1
    )
    sumexp = np.stack(
        [np.asarray(res.results[c]["sumexp_out"]) for c in range(NCORES)], axis=0
    ).sum(axis=0)

    dec_out = np.concatenate(
        [np.zeros((1, B, V), np.float32), logits.reshape(T - 1, B, V)], axis=0
    )
    tgt = np.asarray(f["y"])[1:].reshape(-1).astype(np.int64)
    logz = np.log(sumexp)
    nll = logz - logits[np.arange(NTOK_Y), tgt]
    mask = (tgt != 0).astype(np.float32)
    loss = np.float32((nll * mask).sum() / max(mask.sum(), 1.0))
    return dec_out, loss
